# revision 26
# baseline (speedup 1.0000x reference)
"""Trainium2 Bass kernel for nn_Encoder_61177514164477 (meta-GCN LSTM encoder).

Sharding: 8 cores = 4 batch groups x 2 node-halves. Core c handles batch
b = c%4 and node rows [half*1024, (half+1)*1024) with half = c//4.

v2: fp8 (TRN E4M3) DoubleRow einsum1 — G^T tiles and the LSTM states are
stored fp8 (G x 2^17, h x 2^7; the 2^24 product scale is divided out of the
host-side einsum2 weights). Layer-0 einsum2 uses 128/71-deep stacked
stationaries ([P.k0|P.k1] and [P.k2|gx|ones]) built by psum-evac copies +
one partition-shift DMA, halving its matmul count. The h1-init exchange is
host-built (no startup collective); the per-step h exchanges are pairwise
masked ReduceScatters on fp8 payloads.
"""
import numpy as np
import ml_dtypes

import concourse.bass as bass
import concourse.mybir as mybir
import concourse.tile as tile
import concourse.bacc as bacc
import concourse.tile_utils as tile_utils
from concourse.bass_utils import run_bass_kernel_spmd

tile_utils.max_sbuf_usage = 204 * 1024

L, B, T, N, C, H, K, M = 2, 4, 8, 2048, 2, 64, 3, 32
DIN0, DIN1, DOUT = C + H, 2 * H, 4 * H
HALF = N // 2          # 1024 rows per core
JT = N // 128          # 16 j-tiles (local order: 8 own + 8 partner)
JTP = JT // 2          # 8 j-tile pairs (DoubleRow packs 2 j-tiles/matmul)
IT = HALF // 128       # 8 own i-tiles
NCORES = 8
PAIRS = [[0, 4], [1, 5], [2, 6], [3, 7]]
XR = 2 * K + 1         # rows of the gx block: (k,c) pairs + ones row
BROWS = 64 + XR        # B-stationary depth: P.k2 rows + gx rows

GS = float(2 ** 17)    # G fp8 scale
HS = float(2 ** 7)     # h fp8 scale
PS = GS * HS           # scale carried by P (divided out of w0hA/wBk2/w1)

F32 = mybir.dt.float32
BF16 = mybir.dt.bfloat16
FP8 = mybir.dt.float8e4
BF = ml_dtypes.bfloat16
F8 = ml_dtypes.float8_e4m3   # TRN-style E4M3 (max 240, has inf)

_CACHE = {}
LAST_RESULT = None


def _build():
    if "nc" in _CACHE:
        return _CACHE["nc"]
    nc = bacc.Bacc(None, target_bir_lowering=False, debug=False)

    # ---- external inputs (host-prepped layouts) ----
    c0_in = nc.declare_dram_parameter("c0_init", [128, IT * H], F32, isOutput=False)
    c1_in = nc.declare_dram_parameter("c1_init", [128, IT * H], F32, isOutput=False)
    s1v_in = nc.declare_dram_parameter("s1v_init", [128, JTP * 2 * DIN1], FP8,
                                       isOutput=False)
    m64_in = nc.declare_dram_parameter("mask64", [128, 2], F32, isOutput=False)
    sA_in = nc.declare_dram_parameter("s0hA", [128, HALF], BF16, isOutput=False)
    sB_in = nc.declare_dram_parameter("s0hB", [128, HALF], BF16, isOutput=False)
    wA_in = nc.declare_dram_parameter("wA", [128, T * DOUT], BF16, isOutput=False)
    wB_in = nc.declare_dram_parameter("wB", [BROWS, T * DOUT], BF16, isOutput=False)
    gxb_in = nc.declare_dram_parameter("gxb", [XR, T * HALF], BF16, isOutput=False)
    gt_in = nc.declare_dram_parameter("gt8", [JTP, 128, K * 2 * HALF], FP8,
                                      isOutput=False)
    h1s_in = nc.declare_dram_parameter("hst1_init", [128, 2 * IT * H], FP8,
                                       isOutput=False)
    w1_in = nc.declare_dram_parameter("w1", [DIN1, T * K * DOUT], BF16, isOutput=False)
    b1_in = nc.declare_dram_parameter("bias1", [128, T * DOUT], BF16, isOutput=False)
    out_ext = nc.declare_dram_parameter("out", [2, L, 128, IT * H], F32, isOutput=True)

    MULT = mybir.AluOpType.mult
    ADD = mybir.AluOpType.add
    SIG = mybir.ActivationFunctionType.Sigmoid
    TANH = mybir.ActivationFunctionType.Tanh
    DR = mybir.MatmulPerfMode.DoubleRow

    with tile.TileContext(nc) as tc:
        with tc.tile_pool(name="const", bufs=1) as cpool, \
             tc.tile_pool(name="stat", bufs=2) as spool, \
             tc.tile_pool(name="work", bufs=1) as wpool, \
             tc.tile_pool(name="psum", bufs=1, space="PSUM") as ppool, \
             tc.tile_pool(name="dram", bufs=1, space="DRAM") as dpool:

            # ---- constants, DMA order = arrival priority ----
            c_all = []
            for l, cin in ((0, c0_in), (1, c1_in)):
                ct = cpool.tile([128, IT * H], F32, name=f"c{l}_all", tag=f"c{l}_all")
                nc.sync.dma_start(ct[:], cin[:])
                c_all.append(ct)
            m64_sb = cpool.tile([128, 2], F32, name="m64_sb", tag="m64_sb")
            nc.sync.dma_start(m64_sb[:], m64_in[:])
            mk64 = [m64_sb[:, 0:1], m64_sb[:, 1:2]]

            # state tiles (fp8): [p, jtp, e, feat]; h0 = feat 0:64, h1 = 64:128
            stat_cur = spool.tile([128, JTP * 2 * DIN1], FP8, name="stat1", tag="stat1")
            nc.sync.dma_start(stat_cur[:], s1v_in[:])
            s1v = stat_cur[:].rearrange("p (jtp e f) -> p jtp e f", e=2, f=DIN1)

            # l0 stationaries (double-buffered across steps)
            A_t = spool.tile([128, HALF], BF16, name="A_t", tag="A_t")
            nc.sync.dma_start(A_t[:], sA_in[:])
            B_t = spool.tile([128, HALF], BF16, name="B_t", tag="B_t")
            nc.sync.dma_start(B_t[:], sB_in[:])
            wA_sb = cpool.tile([128, T * DOUT], BF16, name="wA_sb", tag="wA_sb")
            nc.sync.dma_start(wA_sb[:], wA_in[:])
            wB_sb = cpool.tile([BROWS, T * DOUT], BF16, name="wB_sb", tag="wB_sb")
            nc.sync.dma_start(wB_sb[:], wB_in[:])

            # ---- G^T fp8 tiles: own j-pairs first (e1 own chases these) ----
            gt_sb = []
            for jp in range(JTP):
                t_ = cpool.tile([128, K * 2 * HALF], FP8, name=f"gt{jp}",
                                tag=f"gt{jp}")
                nc.sync.dma_start(t_[:], gt_in[jp])
                gt_sb.append(t_)
            gt_v = [t_[:].rearrange("p (k e i) -> p k e i", k=K, e=2)
                    for t_ in gt_sb]

            w1_sb = cpool.tile([DIN1, T * K * DOUT], BF16, name="w1_sb", tag="w1_sb")
            nc.sync.dma_start(w1_sb[:], w1_in[:])
            b1_sb = cpool.tile([128, T * DOUT], BF16, name="b1_sb", tag="b1_sb")
            nc.sync.dma_start(b1_sb[:], b1_in[:])

            # dram bounce/output buffers per parity; one combined RS per step
            # carries [h0_t | h1_{t-1}] for all own j-tiles
            bounce = [dpool.tile([2, 128, IT * DIN1], FP8, name=f"bounce{i}",
                                 tag=f"bounce{i}") for i in range(2)]
            rs_out = [dpool.tile([128, IT * DIN1], FP8, name=f"rso{i}",
                                 tag=f"rso{i}") for i in range(2)]

            # fp8 send staging per parity: [p, slot, it, DIN1]
            hstage = [wpool.tile([128, 2 * IT * DIN1], FP8, name=f"hst{i}",
                                 tag=f"hst{i}") for i in range(2)]
            hs_v = [h[:].rearrange("p (s it f) -> p s it f", s=2, f=DIN1)
                    for h in hstage]
            # t=0 RS carries the (masked) host h1-init in its h1 columns
            nc.scalar.dma_start(
                hs_v[0][:, :, :, H:DIN1],
                h1s_in[:].rearrange("p (s it f) -> p s it f", s=2, f=H))

            def send_h(tslot):
                for s in range(2):
                    nc.sync.dma_start(bounce[tslot][s],
                                      hs_v[tslot][:, s].rearrange("p it f -> p (it f)"))
                nc.gpsimd.collective_compute(
                    "ReduceScatter", mybir.AluOpType.add, replica_groups=PAIRS,
                    ins=[bounce[tslot].opt()], outs=[rs_out[tslot].opt()],
                )

            hf1 = wpool.tile([128, IT * H], F32, name="hf1", tag="hf1")
            hf0 = wpool.tile([128, IT * H], F32, name="hf0", tag="hf0")

            def gates(conv_all, ih, l, t):
                """LSTM gates on half ih: conv [128, 4it x 4gates x 64].

                Writes c in place; h goes (x2^7) to s1v/hstage slots as fp8,
                and unscaled to hf0/hf1 on the last step.
                """
                cv = conv_all[:, ih * 4 * DOUT:(ih + 1) * 4 * DOUT].rearrange(
                    "p (it g c) -> p it g c", g=4, c=H)
                HB = 4 * H
                tg = wpool.tile([128, HB], BF16, name="g_tg", tag="g_tg", bufs=2)
                sg3 = wpool.tile([128, 3 * HB], BF16, name="g_s3", tag="g_s3", bufs=2)
                sgv = sg3[:].rearrange("p (it g c) -> p it g c", g=3, c=H)
                nc.scalar.activation(tg[:], cv[:, :, 3, :], TANH)
                nc.scalar.activation(sgv, cv[:, :, 0:3, :], SIG)
                m1 = wpool.tile([128, HB], F32, name="g_m1", tag="g_m1", bufs=1)
                m2 = wpool.tile([128, HB], F32, name="g_m2", tag="g_m2", bufs=1)
                ch = c_all[l][:, ih * HB:(ih + 1) * HB]
                tg_v = tg[:].rearrange("p (it c) -> p it c", c=H)
                ch_v = ch.rearrange("p (it c) -> p it c", c=H)
                nc.vector.tensor_tensor(
                    m2[:].rearrange("p (it c) -> p it c", c=H),
                    sgv[:, :, 0, :], tg_v, MULT)
                nc.vector.tensor_tensor(
                    m1[:].rearrange("p (it c) -> p it c", c=H),
                    sgv[:, :, 1, :], ch_v, MULT)
                nc.vector.tensor_tensor(ch, m1[:], m2[:], ADD)
                tanh_c = wpool.tile([128, HB], BF16, name="g_tc", tag="g_tc", bufs=2)
                nc.scalar.activation(tanh_c[:], ch, TANH)
                so = sgv[:, :, 2, :]
                last = (t == T - 1)
                tg4 = tanh_c[:].rearrange("p (it c) -> p it c", c=H)
                if not (last and l == 1):
                    # own state write (x 2^7, fp8): view [p, jtp2, e2, H]
                    dst = s1v[:, ih * 2:(ih + 1) * 2, :, l * H:(l + 1) * H] \
                        .rearrange("p jtp e f -> p (jtp e) f")
                    nc.vector.scalar_tensor_tensor(dst, so, HS, tg4, MULT, MULT)
                    # masked send slots (x 2^7 x mask); l1 rides next step's RS
                    par = (t + l) % 2
                    for s in range(2):
                        dst = hs_v[par][:, s, ih * 4:(ih + 1) * 4,
                                        l * H:(l + 1) * H]
                        nc.vector.scalar_tensor_tensor(dst, so, mk64[s], tg4,
                                                       MULT, MULT)
                if last:
                    hf = hf0 if l == 0 else hf1
                    nc.vector.tensor_tensor(
                        hf[:, ih * HB:(ih + 1) * HB].rearrange(
                            "p (it c) -> p it c", c=H), so, tg4, MULT)

            def e2_l0(t, At, Bt, conv0):
                """conv0[:, it] = At.T @ wA + Bt[0:71].T @ wB (stacked hops)."""
                for it in range(IT):
                    pc = ppool.tile([128, DOUT], F32, name="e2p", tag="e2p",
                                    bufs=2)
                    nc.tensor.matmul(
                        pc[:], At[:, it * 128:(it + 1) * 128],
                        wA_sb[:, t * DOUT:(t + 1) * DOUT],
                        start=True, stop=False)
                    nc.tensor.matmul(
                        pc[:], Bt[0:BROWS, it * 128:(it + 1) * 128],
                        wB_sb[:, t * DOUT:(t + 1) * DOUT],
                        start=False, stop=True)
                    dst = conv0[:, it * DOUT:(it + 1) * DOUT]
                    if it % 2 == 0:
                        nc.vector.tensor_copy(dst, pc[:])
                    else:
                        nc.scalar.copy(dst, pc[:])

            def e1_mm(psumP, s1v_cur, jp, start, stop):
                lhs = s1v_cur[:, jp]
                for ih in range(2):
                    for k in range(K):
                        nc.tensor.matmul(
                            psumP[k][ih][:], lhs,
                            gt_v[jp][:, k, :, ih * 512:ih * 512 + 512],
                            start=start, stop=stop, perf_mode=DR,
                        )

            def e2_l1_its(t, supP, conv1, its):
                for it in its:
                    pc = ppool.tile([128, DOUT], F32, name="e2p", tag="e2p",
                                    bufs=2)
                    for k in range(K):
                        nc.tensor.matmul(
                            pc[:],
                            supP[k][:, it * 128:(it + 1) * 128],
                            w1_sb[:, (t * K + k) * DOUT:(t * K + k + 1) * DOUT],
                            start=(k == 0), stop=(k == K - 1),
                        )
                    dst = conv1[:, it * DOUT:(it + 1) * DOUT]
                    nc.vector.tensor_tensor(
                        dst, pc[:], b1_sb[:, t * DOUT:(t + 1) * DOUT], ADD)

            # step-0 layer-0 einsum2 peeled to kernel start (host stationaries)
            conv0 = wpool.tile([128, IT * DOUT], BF16, name="conv0", tag="conv0",
                               bufs=2)
            e2_l0(0, A_t[:], B_t[:], conv0)
            for t in range(T):
                # ---------------- layer 0: gates + send ----------------
                for ih in range(2):
                    gates(conv0, ih, 0, t)
                send_h(t % 2)

                # ---------------- einsum1 (fp8 DoubleRow) ------------------
                psumP = [[ppool.tile([128, 512], F32, name=f"e1p{k}{ih}",
                                     tag=f"e1p{k}{ih}", bufs=1)
                          for ih in range(2)] for k in range(K)]
                for jp in range(4):
                    e1_mm(psumP, s1v, jp, jp == 0, False)
                # full partner state block arrives in one contiguous DMA
                nc.sync.dma_start(
                    s1v[:, 4:8, :, :].rearrange("p jtp e f -> p (jtp e f)"),
                    rs_out[t % 2][:])
                for jp in range(4, 8):
                    e1_mm(psumP, s1v, jp, False, jp == 7)

                # ---------------- evac P + next-step l0 stationaries -------
                supP = [wpool.tile([128, HALF], BF16, name=f"supP{k}",
                                   tag=f"supP{k}", bufs=2) for k in range(K)]
                conv1 = wpool.tile([128, IT * DOUT], BF16, name="conv1", tag="conv1")
                if t + 1 < T:
                    A_nxt = spool.tile([128, HALF], BF16, name="A_t", tag="A_t")
                    B_nxt = spool.tile([128, HALF], BF16, name="B_t", tag="B_t")
                    # gx rows of B for step t+1 straight from DRAM
                    nc.scalar.dma_start(
                        B_nxt[64:BROWS, :],
                        gxb_in[:, (t + 1) * HALF:(t + 2) * HALF])
                for ih in range(2):
                    for k in range(K):
                        dst = supP[k][:, ih * 512:(ih + 1) * 512]
                        if (k + ih) % 2 == 0:
                            nc.vector.tensor_copy(dst, psumP[k][ih][:])
                        else:
                            nc.scalar.copy(dst, psumP[k][ih][:])
                    if t + 1 < T:
                        lo, hi = ih * 512, (ih + 1) * 512
                        nc.vector.tensor_copy(A_nxt[0:64, lo:hi],
                                              psumP[0][ih][0:64, :])
                        nc.scalar.copy(B_nxt[0:64, lo:hi],
                                       psumP[2][ih][0:64, :])
                        # partition shift 0:64 -> 64:128 via SBUF-SBUF DMA
                        nc.sync.dma_start(A_nxt[64:128, lo:hi],
                                          supP[1][0:64, lo:hi])
                    # layer-1 einsum2 for this half right away
                    e2_l1_its(t, supP, conv1, range(ih * 4, ih * 4 + 4))

                # ------- next step's layer-0 einsum2 before l1 gates -------
                if t + 1 < T:
                    conv0 = wpool.tile([128, IT * DOUT], BF16, name="conv0",
                                       tag="conv0", bufs=2)
                    e2_l0(t + 1, A_nxt[:], B_nxt[:], conv0)
                    stat_nxt = spool.tile([128, JTP * 2 * DIN1], FP8, name="stat1",
                                          tag="stat1")
                    s1v = stat_nxt[:].rearrange("p (jtp e f) -> p jtp e f",
                                                e=2, f=DIN1)
                # ---------------- layer 1: gates ----------------
                for ih in range(2):
                    gates(conv1, ih, 1, t)

            # ---------------- outputs ----------------
            # final h0 (t=T-1 wrote fp8 x128 into s1v; hf0 got the clean copy)
            nc.sync.dma_start(out_ext[0, 0], hf0[:])
            nc.sync.dma_start(out_ext[0, 1], hf1[:])
            nc.sync.dma_start(out_ext[1, 0], c_all[0][:])
            nc.sync.dma_start(out_ext[1, 1], c_all[1][:])

    nc.compile()
    _CACHE["nc"] = nc
    return nc


def _host_prep(inputs):
    """Per-core input maps (all device layouts built here)."""
    G = np.asarray(inputs["G"], np.float32)
    x_seq = np.asarray(inputs["x_seq"], np.float32)
    init_h = np.asarray(inputs["init_h"], np.float32)
    init_c = np.asarray(inputs["init_c"], np.float32)
    x_meta = np.asarray(inputs["x_meta"], np.float32)

    def mlp(b, w1, b1, w2, b2):
        hid = np.maximum(x_meta[b] @ w1 + b1, 0.0)
        return hid @ w2 + b2

    GF = G.reshape(K * N, N)
    in_maps = []
    for c in range(NCORES):
        b, half = c % 4, c // 4
        own = np.arange(half * HALF, (half + 1) * HALF)
        par = np.arange((1 - half) * HALF, (2 - half) * HALF)
        jperm = np.concatenate([own, par])

        # GT fp8: [JTP, 128, K, 2, HALF]; pair e = j-tile 2*jp+e
        gt = G[:, jperm, :][:, :, own].transpose(1, 0, 2)      # [N, K, HALF]
        gt = gt.reshape(JTP, 2, 128, K, HALF).transpose(0, 2, 3, 1, 4)
        gt8 = np.ascontiguousarray(
            (gt * GS).reshape(JTP, 128, K * 2 * HALF)).astype(F8)

        # host Gx rows for the B stationary (k,c) pairs + ones row
        xb = np.ascontiguousarray(x_seq[b].transpose(1, 0, 2).reshape(N, T * C))
        gx = (GF @ xb).reshape(K, N, T, C)
        gxb = np.zeros((XR, T * HALF), np.float32)
        for k in range(K):
            for cc in range(C):
                gxb[k * C + cc] = gx[k, own, :, cc].T.reshape(T * HALF)
        gxb[XR - 1] = 1.0
        gxb = gxb.astype(BF)

        # host G.h0_init stacked into the A/B stationaries (x PS)
        s0 = np.zeros((K, 64, HALF), np.float32)
        if init_h[0, b].any():
            gh = (GF @ init_h[0, b]).reshape(K, N, H)
            s0 = np.ascontiguousarray(gh[:, own, :].transpose(0, 2, 1)) * PS
        s0hA = np.concatenate([s0[0], s0[1]], axis=0).astype(BF)       # [128, HALF]
        s0hB = np.zeros((128, HALF), np.float32)
        s0hB[0:64] = s0[2]
        s0hB[64:64 + XR] = gxb[:, 0:HALF].astype(np.float32)           # t=0 gx rows
        s0hB = s0hB.astype(BF)

        # layer-0 weights: A rows = [k0 h | k1 h] (/PS); B rows = [k2 h (/PS),
        # gx (k,c) pairs, bias row]
        W0 = mlp(b, inputs["lw1_0"], inputs["lb1_0"], inputs["lw2_0"], inputs["lb2_0"])
        W0 = np.asarray(W0, np.float32).reshape(T, K, DIN0, DOUT)
        bias0 = np.asarray(
            mlp(b, inputs["bw1_0"], inputs["bb1_0"], inputs["bw2_0"], inputs["bb2_0"]),
            np.float32).reshape(T, DOUT)
        wA = np.concatenate([W0[:, 0, C:, :], W0[:, 1, C:, :]],
                            axis=1) / PS                                # [T,128,DOUT]
        wA = wA.transpose(1, 0, 2).reshape(128, T * DOUT).astype(BF)
        wB = np.zeros((BROWS, T, DOUT), np.float32)
        wB[0:64] = W0[:, 2, C:, :].transpose(1, 0, 2) / PS
        for k in range(K):
            for cc in range(C):
                wB[64 + k * C + cc] = W0[:, k, cc, :]
        wB[64 + XR - 1] = bias0
        wB = wB.reshape(BROWS, T * DOUT).astype(BF)

        W1 = mlp(b, inputs["lw1_1"], inputs["lb1_1"], inputs["lw2_1"], inputs["lb2_1"])
        W1 = np.asarray(W1, np.float32).reshape(T, K, DIN1, DOUT) / PS
        w1 = W1.transpose(2, 0, 1, 3).reshape(DIN1, T * K * DOUT).astype(BF)
        bias1 = np.asarray(
            mlp(b, inputs["bw1_1"], inputs["bb1_1"], inputs["bw2_1"], inputs["bb2_1"]),
            np.float32)
        b1 = np.ascontiguousarray(
            np.broadcast_to(bias1.reshape(1, T * DOUT), (128, T * DOUT))).astype(BF)

        c0 = np.ascontiguousarray(
            init_c[0, b][own].reshape(IT, 128, H).transpose(1, 0, 2).reshape(128, IT * H))
        c1 = np.ascontiguousarray(
            init_c[1, b][own].reshape(IT, 128, H).transpose(1, 0, 2).reshape(128, IT * H))

        # fp8 state init: [p, jtp, e, feat]; h1 rows filled (x HS), h0 zero
        s1v0 = np.zeros((JTP, 2, 128, DIN1), np.float32)
        h1o = init_h[1, b][jperm].reshape(JTP, 2, 128, H) * HS
        s1v0[:, :, :, H:DIN1] = h1o
        s1v0 = np.ascontiguousarray(
            s1v0.transpose(2, 0, 1, 3).reshape(128, JTP * 2 * DIN1)).astype(F8)

        # masked h1-init send slots for the t=0 combined RS: [p, s, it, H]
        h1own = init_h[1, b][own].reshape(IT, 128, H) * HS
        hst1 = np.zeros((2, IT, 128, H), np.float32)
        hst1[0] = h1own * (1 - half)
        hst1[1] = h1own * half
        hst1 = np.ascontiguousarray(
            hst1.transpose(2, 0, 1, 3).reshape(128, 2 * IT * H)).astype(F8)

        in_maps.append({
            "gt8": gt8,
            "s0hA": np.ascontiguousarray(s0hA),
            "s0hB": np.ascontiguousarray(s0hB),
            "wA": np.ascontiguousarray(wA),
            "wB": np.ascontiguousarray(wB),
            "gxb": np.ascontiguousarray(gxb),
            "w1": np.ascontiguousarray(w1),
            "bias1": b1,
            "c0_init": np.ascontiguousarray(c0, np.float32),
            "c1_init": np.ascontiguousarray(c1, np.float32),
            "s1v_init": s1v0,
            "hst1_init": hst1,
            "mask64": np.ascontiguousarray(np.broadcast_to(
                np.array([(1 - half) * HS, half * HS], np.float32).reshape(1, 2),
                (128, 2))),
        })
    return in_maps


def kernel(**inputs) -> np.ndarray:
    global LAST_RESULT
    nc = _build()
    in_maps = _host_prep(inputs)
    res = run_bass_kernel_spmd(nc, in_maps, list(range(NCORES)))
    LAST_RESULT = res

    out = np.zeros((2, L, B, N, H), np.float32)
    for c in range(NCORES):
        b, half = c % 4, c // 4
        o = res.results[c]["out"].reshape(2, L, 128, IT, H)
        out[:, :, b, half * HALF:(half + 1) * HALF, :] = o.transpose(0, 1, 3, 2, 4).reshape(
            2, L, HALF, H)
    return out


# revision 30
# speedup vs baseline: 1.2168x; 1.2168x over previous
"""Trainium2 Bass kernel for nn_Encoder_61177514164477 (meta-GCN LSTM encoder).

Sharding: 8 cores = 4 batch groups x 2 node-halves. Core c handles batch
b = c%4 and node rows [half*1024, (half+1)*1024) with half = c//4.

v2: fp8 (TRN E4M3) DoubleRow einsum1 — G^T tiles and the LSTM states are
stored fp8 (G x 2^17, h x 2^7; the 2^24 product scale is divided out of the
host-side einsum2 weights). Layer-0 einsum2 uses 128/71-deep stacked
stationaries ([P.k0|P.k1] and [P.k2|gx|ones]) built by psum-evac copies +
one partition-shift DMA, halving its matmul count. The h1-init exchange is
host-built (no startup collective); the per-step h exchanges are pairwise
masked ReduceScatters on fp8 payloads.
"""
import numpy as np
import ml_dtypes

import concourse.bass as bass
import concourse.mybir as mybir
import concourse.tile as tile
import concourse.bacc as bacc
import concourse.tile_utils as tile_utils
from concourse.bass_utils import run_bass_kernel_spmd

tile_utils.max_sbuf_usage = 204 * 1024

L, B, T, N, C, H, K, M = 2, 4, 8, 2048, 2, 64, 3, 32
DIN0, DIN1, DOUT = C + H, 2 * H, 4 * H
HALF = N // 2          # 1024 rows per core
JT = N // 128          # 16 j-tiles (local order: 8 own + 8 partner)
JTP = JT // 2          # 8 j-tile pairs (DoubleRow packs 2 j-tiles/matmul)
IT = HALF // 128       # 8 own i-tiles
NCORES = 8
PAIRS = [[0, 4], [1, 5], [2, 6], [3, 7]]
XR = 2 * K + 1         # rows of the gx block: (k,c) pairs + ones row
BROWS = 64 + XR        # B-stationary depth: P.k2 rows + gx rows

GS = float(2 ** 17)    # G fp8 scale
HS = float(2 ** 7)     # h fp8 scale
PS = GS * HS           # scale carried by P (divided out of w0hA/wBk2/w1)

F32 = mybir.dt.float32
BF16 = mybir.dt.bfloat16
FP8 = mybir.dt.float8e4
BF = ml_dtypes.bfloat16
F8 = ml_dtypes.float8_e4m3   # TRN-style E4M3 (max 240, has inf)

_CACHE = {}
LAST_RESULT = None


def _build():
    if "nc" in _CACHE:
        return _CACHE["nc"]
    nc = bacc.Bacc(None, target_bir_lowering=False, debug=False)

    # ---- external inputs (host-prepped layouts) ----
    c0_in = nc.declare_dram_parameter("c0_init", [128, IT * H], F32, isOutput=False)
    c1_in = nc.declare_dram_parameter("c1_init", [128, IT * H], F32, isOutput=False)
    s1v_in = nc.declare_dram_parameter("s1v_init", [128, JTP * 2 * DIN1], FP8,
                                       isOutput=False)
    m64_in = nc.declare_dram_parameter("mask64", [128, 2], F32, isOutput=False)
    sA_in = nc.declare_dram_parameter("s0hA", [128, HALF], BF16, isOutput=False)
    sB_in = nc.declare_dram_parameter("s0hB", [128, HALF], BF16, isOutput=False)
    wA_in = nc.declare_dram_parameter("wA", [128, T * DOUT], BF16, isOutput=False)
    wB_in = nc.declare_dram_parameter("wB", [BROWS, T * DOUT], BF16, isOutput=False)
    gxb_in = nc.declare_dram_parameter("gxb", [XR, T * HALF], BF16, isOutput=False)
    gt_in = nc.declare_dram_parameter("gt8", [JTP, 128, K * 2 * HALF], FP8,
                                      isOutput=False)
    w1_in = nc.declare_dram_parameter("w1", [DIN1, T * K * DOUT], BF16, isOutput=False)
    b1_in = nc.declare_dram_parameter("bias1", [128, T * DOUT], BF16, isOutput=False)
    out_ext = nc.declare_dram_parameter("out", [2, L, 128, IT * H], F32, isOutput=True)

    MULT = mybir.AluOpType.mult
    ADD = mybir.AluOpType.add
    SIG = mybir.ActivationFunctionType.Sigmoid
    TANH = mybir.ActivationFunctionType.Tanh
    DR = mybir.MatmulPerfMode.DoubleRow

    with tile.TileContext(nc) as tc:
        with tc.tile_pool(name="const", bufs=1) as cpool, \
             tc.tile_pool(name="stat", bufs=2) as spool, \
             tc.tile_pool(name="work", bufs=1) as wpool, \
             tc.tile_pool(name="psum", bufs=1, space="PSUM") as ppool, \
             tc.tile_pool(name="dram", bufs=1, space="DRAM") as dpool:

            # ---- constants, DMA order = arrival priority ----
            c_all = []
            for l, cin in ((0, c0_in), (1, c1_in)):
                ct = cpool.tile([128, IT * H], F32, name=f"c{l}_all", tag=f"c{l}_all")
                nc.sync.dma_start(ct[:], cin[:])
                c_all.append(ct)
            m64_sb = cpool.tile([128, 2], F32, name="m64_sb", tag="m64_sb")
            nc.sync.dma_start(m64_sb[:], m64_in[:])
            mkh = [m64_sb[:, 0:1], m64_sb[:, 1:2]]

            # state tiles (fp8): [p, jtp, e, feat]; h0 = feat 0:64, h1 = 64:128
            stat_cur = spool.tile([128, JTP * 2 * DIN1], FP8, name="stat1", tag="stat1")
            nc.sync.dma_start(stat_cur[:], s1v_in[:])
            s1v = stat_cur[:].rearrange("p (jtp e f) -> p jtp e f", e=2, f=DIN1)

            # l0 stationaries (double-buffered across steps)
            A_t = spool.tile([128, HALF], BF16, name="A_t", tag="A_t")
            nc.sync.dma_start(A_t[:], sA_in[:])
            B_t = spool.tile([128, HALF], BF16, name="B_t", tag="B_t")
            nc.sync.dma_start(B_t[:], sB_in[:])
            wA_sb = cpool.tile([128, T * DOUT], BF16, name="wA_sb", tag="wA_sb")
            nc.sync.dma_start(wA_sb[:], wA_in[:])
            wB_sb = cpool.tile([BROWS, T * DOUT], BF16, name="wB_sb", tag="wB_sb")
            nc.sync.dma_start(wB_sb[:], wB_in[:])

            # ---- G^T fp8 tiles: own j-pairs first (e1 own chases these) ----
            gt_sb = []
            for jp in range(JTP):
                t_ = cpool.tile([128, K * 2 * HALF], FP8, name=f"gt{jp}",
                                tag=f"gt{jp}")
                nc.sync.dma_start(t_[:], gt_in[jp])
                gt_sb.append(t_)
            gt_v = [t_[:].rearrange("p (k e i) -> p k e i", k=K, e=2)
                    for t_ in gt_sb]

            w1_sb = cpool.tile([DIN1, T * K * DOUT], BF16, name="w1_sb", tag="w1_sb")
            nc.sync.dma_start(w1_sb[:], w1_in[:])
            b1_sb = cpool.tile([128, T * DOUT], BF16, name="b1_sb", tag="b1_sb")
            nc.sync.dma_start(b1_sb[:], b1_in[:])

            # dram bounce/output buffers per parity; one combined AllGather
            # per step carries [h0_t | h1_{t-1}] for the own j-tiles
            bounce = [dpool.tile([128, IT * DIN1], FP8, name=f"bounce{i}",
                                 tag=f"bounce{i}") for i in range(2)]
            ag_out = [dpool.tile([2, 128, IT * DIN1], FP8, name=f"ago{i}",
                                 tag=f"ago{i}") for i in range(2)]
            # chunk staging in SBUF for the masked combine
            astg = [wpool.tile([128, IT * DIN1], FP8, name=f"astg{s}",
                               tag=f"astg{s}", bufs=2) for s in range(2)]

            def send_h(tslot, own_flat):
                nc.sync.dma_start(bounce[tslot][:], own_flat)
                nc.gpsimd.collective_compute(
                    "AllGather", mybir.AluOpType.bypass, replica_groups=PAIRS,
                    ins=[bounce[tslot].opt()], outs=[ag_out[tslot].opt()],
                )

            def recv_h(tslot, dst_flat):
                """dst = chunk0*half + chunk1*(1-half): picks the partner."""
                a0 = astg[0][:]
                a1 = astg[1][:]
                nc.sync.dma_start(a0, ag_out[tslot][0])
                nc.scalar.dma_start(a1, ag_out[tslot][1])
                tmp = wpool.tile([128, IT * DIN1], FP8, name="cmb", tag="cmb",
                                 bufs=2)
                nc.vector.tensor_scalar_mul(tmp[:], a0, mkh[1])
                nc.vector.scalar_tensor_tensor(dst_flat, a1, mkh[0], tmp[:],
                                               MULT, ADD)

            hf1 = wpool.tile([128, IT * H], F32, name="hf1", tag="hf1")
            hf0 = wpool.tile([128, IT * H], F32, name="hf0", tag="hf0")

            def gates(conv_all, ih, l, t):
                """LSTM gates on half ih: conv [128, 4it x 4gates x 64].

                Writes c in place; h goes (x2^7) to s1v/hstage slots as fp8,
                and unscaled to hf0/hf1 on the last step.
                """
                cv = conv_all[:, ih * 4 * DOUT:(ih + 1) * 4 * DOUT].rearrange(
                    "p (it g c) -> p it g c", g=4, c=H)
                HB = 4 * H
                tg = wpool.tile([128, HB], BF16, name="g_tg", tag="g_tg", bufs=2)
                sg3 = wpool.tile([128, 3 * HB], BF16, name="g_s3", tag="g_s3", bufs=2)
                sgv = sg3[:].rearrange("p (it g c) -> p it g c", g=3, c=H)
                nc.scalar.activation(tg[:], cv[:, :, 3, :], TANH)
                nc.scalar.activation(sgv, cv[:, :, 0:3, :], SIG)
                m1 = wpool.tile([128, HB], F32, name="g_m1", tag="g_m1", bufs=1)
                m2 = wpool.tile([128, HB], F32, name="g_m2", tag="g_m2", bufs=1)
                ch = c_all[l][:, ih * HB:(ih + 1) * HB]
                tg_v = tg[:].rearrange("p (it c) -> p it c", c=H)
                ch_v = ch.rearrange("p (it c) -> p it c", c=H)
                nc.vector.tensor_tensor(
                    m2[:].rearrange("p (it c) -> p it c", c=H),
                    sgv[:, :, 0, :], tg_v, MULT)
                nc.vector.tensor_tensor(
                    m1[:].rearrange("p (it c) -> p it c", c=H),
                    sgv[:, :, 1, :], ch_v, MULT)
                nc.vector.tensor_tensor(ch, m1[:], m2[:], ADD)
                tanh_c = wpool.tile([128, HB], BF16, name="g_tc", tag="g_tc", bufs=2)
                nc.scalar.activation(tanh_c[:], ch, TANH)
                so = sgv[:, :, 2, :]
                last = (t == T - 1)
                tg4 = tanh_c[:].rearrange("p (it c) -> p it c", c=H)
                if not (last and l == 1):
                    # own state write (x 2^7, fp8): view [p, jtp2, e2, H]
                    dst = s1v[:, ih * 2:(ih + 1) * 2, :, l * H:(l + 1) * H] \
                        .rearrange("p jtp e f -> p (jtp e) f")
                    nc.vector.scalar_tensor_tensor(dst, so, HS, tg4, MULT, MULT)
                if last:
                    hf = hf0 if l == 0 else hf1
                    nc.vector.tensor_tensor(
                        hf[:, ih * HB:(ih + 1) * HB].rearrange(
                            "p (it c) -> p it c", c=H), so, tg4, MULT)

            def e2_l0(t, At, Bt, conv0):
                """conv0[:, it] = At.T @ wA + Bt[0:71].T @ wB (stacked hops)."""
                for it in range(IT):
                    pc = ppool.tile([128, DOUT], F32, name="e2p", tag="e2p",
                                    bufs=2)
                    nc.tensor.matmul(
                        pc[:], At[:, it * 128:(it + 1) * 128],
                        wA_sb[:, t * DOUT:(t + 1) * DOUT],
                        start=True, stop=False)
                    nc.tensor.matmul(
                        pc[:], Bt[0:BROWS, it * 128:(it + 1) * 128],
                        wB_sb[:, t * DOUT:(t + 1) * DOUT],
                        start=False, stop=True)
                    dst = conv0[:, it * DOUT:(it + 1) * DOUT]
                    if it % 2 == 0:
                        nc.vector.tensor_copy(dst, pc[:])
                    else:
                        nc.scalar.copy(dst, pc[:])

            def e1_mm(psumP, s1v_cur, jp, start, stop):
                lhs = s1v_cur[:, jp]
                for ih in range(2):
                    for k in range(K):
                        nc.tensor.matmul(
                            psumP[k][ih][:], lhs,
                            gt_v[jp][:, k, :, ih * 512:ih * 512 + 512],
                            start=start, stop=stop, perf_mode=DR,
                        )

            def e2_l1_its(t, supP, conv1, its):
                for it in its:
                    pc = ppool.tile([128, DOUT], F32, name="e2p", tag="e2p",
                                    bufs=2)
                    for k in range(K):
                        nc.tensor.matmul(
                            pc[:],
                            supP[k][:, it * 128:(it + 1) * 128],
                            w1_sb[:, (t * K + k) * DOUT:(t * K + k + 1) * DOUT],
                            start=(k == 0), stop=(k == K - 1),
                        )
                    dst = conv1[:, it * DOUT:(it + 1) * DOUT]
                    nc.vector.tensor_tensor(
                        dst, pc[:], b1_sb[:, t * DOUT:(t + 1) * DOUT], ADD)

            # step-0 layer-0 einsum2 peeled to kernel start (host stationaries)
            conv0 = wpool.tile([128, IT * DOUT], BF16, name="conv0", tag="conv0",
                               bufs=2)
            e2_l0(0, A_t[:], B_t[:], conv0)
            for t in range(T):
                # ---------------- layer 0: gates + send ----------------
                for ih in range(2):
                    gates(conv0, ih, 0, t)
                send_h(t % 2, s1v[:, 0:4, :, :].rearrange("p jtp e f -> p (jtp e f)"))

                # ---------------- einsum1 (fp8 DoubleRow) ------------------
                psumP = [[ppool.tile([128, 512], F32, name=f"e1p{k}{ih}",
                                     tag=f"e1p{k}{ih}", bufs=1)
                          for ih in range(2)] for k in range(K)]
                for jp in range(4):
                    e1_mm(psumP, s1v, jp, jp == 0, False)
                # full partner state block arrives via AG chunks + combine
                recv_h(t % 2, s1v[:, 4:8, :, :].rearrange("p jtp e f -> p (jtp e f)"))
                for jp in range(4, 8):
                    e1_mm(psumP, s1v, jp, False, jp == 7)

                # ---------------- evac P + next-step l0 stationaries -------
                supP = [wpool.tile([128, HALF], BF16, name=f"supP{k}",
                                   tag=f"supP{k}", bufs=2) for k in range(K)]
                conv1 = wpool.tile([128, IT * DOUT], BF16, name="conv1", tag="conv1")
                if t + 1 < T:
                    A_nxt = spool.tile([128, HALF], BF16, name="A_t", tag="A_t")
                    B_nxt = spool.tile([128, HALF], BF16, name="B_t", tag="B_t")
                    # gx rows of B for step t+1 straight from DRAM
                    nc.scalar.dma_start(
                        B_nxt[64:BROWS, :],
                        gxb_in[:, (t + 1) * HALF:(t + 2) * HALF])
                for ih in range(2):
                    for k in range(K):
                        dst = supP[k][:, ih * 512:(ih + 1) * 512]
                        if (k + ih) % 2 == 0:
                            nc.vector.tensor_copy(dst, psumP[k][ih][:])
                        else:
                            nc.scalar.copy(dst, psumP[k][ih][:])
                    if t + 1 < T:
                        lo, hi = ih * 512, (ih + 1) * 512
                        nc.vector.tensor_copy(A_nxt[0:64, lo:hi],
                                              psumP[0][ih][0:64, :])
                        nc.scalar.copy(B_nxt[0:64, lo:hi],
                                       psumP[2][ih][0:64, :])
                        # partition shift 0:64 -> 64:128 via SBUF-SBUF DMA
                        nc.sync.dma_start(A_nxt[64:128, lo:hi],
                                          supP[1][0:64, lo:hi])
                    # layer-1 einsum2 for this half right away
                    e2_l1_its(t, supP, conv1, range(ih * 4, ih * 4 + 4))

                # ------- next step's layer-0 einsum2 before l1 gates -------
                if t + 1 < T:
                    conv0 = wpool.tile([128, IT * DOUT], BF16, name="conv0",
                                       tag="conv0", bufs=2)
                    e2_l0(t + 1, A_nxt[:], B_nxt[:], conv0)
                    stat_nxt = spool.tile([128, JTP * 2 * DIN1], FP8, name="stat1",
                                          tag="stat1")
                    s1v = stat_nxt[:].rearrange("p (jtp e f) -> p jtp e f",
                                                e=2, f=DIN1)
                # ---------------- layer 1: gates ----------------
                for ih in range(2):
                    gates(conv1, ih, 1, t)

            # ---------------- outputs ----------------
            # final h0 (t=T-1 wrote fp8 x128 into s1v; hf0 got the clean copy)
            nc.sync.dma_start(out_ext[0, 0], hf0[:])
            nc.sync.dma_start(out_ext[0, 1], hf1[:])
            nc.sync.dma_start(out_ext[1, 0], c_all[0][:])
            nc.sync.dma_start(out_ext[1, 1], c_all[1][:])

    nc.compile()
    _CACHE["nc"] = nc
    return nc


def _host_prep(inputs):
    """Per-core input maps (all device layouts built here)."""
    G = np.asarray(inputs["G"], np.float32)
    x_seq = np.asarray(inputs["x_seq"], np.float32)
    init_h = np.asarray(inputs["init_h"], np.float32)
    init_c = np.asarray(inputs["init_c"], np.float32)
    x_meta = np.asarray(inputs["x_meta"], np.float32)

    def mlp(b, w1, b1, w2, b2):
        hid = np.maximum(x_meta[b] @ w1 + b1, 0.0)
        return hid @ w2 + b2

    GF = G.reshape(K * N, N)
    in_maps = []
    for c in range(NCORES):
        b, half = c % 4, c // 4
        own = np.arange(half * HALF, (half + 1) * HALF)
        par = np.arange((1 - half) * HALF, (2 - half) * HALF)
        jperm = np.concatenate([own, par])

        # GT fp8: [JTP, 128, K, 2, HALF]; pair e = j-tile 2*jp+e
        gt = G[:, jperm, :][:, :, own].transpose(1, 0, 2)      # [N, K, HALF]
        gt = gt.reshape(JTP, 2, 128, K, HALF).transpose(0, 2, 3, 1, 4)
        gt8 = np.ascontiguousarray(
            (gt * GS).reshape(JTP, 128, K * 2 * HALF)).astype(F8)

        # host Gx rows for the B stationary (k,c) pairs + ones row
        xb = np.ascontiguousarray(x_seq[b].transpose(1, 0, 2).reshape(N, T * C))
        gx = (GF @ xb).reshape(K, N, T, C)
        gxb = np.zeros((XR, T * HALF), np.float32)
        for k in range(K):
            for cc in range(C):
                gxb[k * C + cc] = gx[k, own, :, cc].T.reshape(T * HALF)
        gxb[XR - 1] = 1.0
        gxb = gxb.astype(BF)

        # host G.h0_init stacked into the A/B stationaries (x PS)
        s0 = np.zeros((K, 64, HALF), np.float32)
        if init_h[0, b].any():
            gh = (GF @ init_h[0, b]).reshape(K, N, H)
            s0 = np.ascontiguousarray(gh[:, own, :].transpose(0, 2, 1)) * PS
        s0hA = np.concatenate([s0[0], s0[1]], axis=0).astype(BF)       # [128, HALF]
        s0hB = np.zeros((128, HALF), np.float32)
        s0hB[0:64] = s0[2]
        s0hB[64:64 + XR] = gxb[:, 0:HALF].astype(np.float32)           # t=0 gx rows
        s0hB = s0hB.astype(BF)

        # layer-0 weights: A rows = [k0 h | k1 h] (/PS); B rows = [k2 h (/PS),
        # gx (k,c) pairs, bias row]
        W0 = mlp(b, inputs["lw1_0"], inputs["lb1_0"], inputs["lw2_0"], inputs["lb2_0"])
        W0 = np.asarray(W0, np.float32).reshape(T, K, DIN0, DOUT)
        bias0 = np.asarray(
            mlp(b, inputs["bw1_0"], inputs["bb1_0"], inputs["bw2_0"], inputs["bb2_0"]),
            np.float32).reshape(T, DOUT)
        wA = np.concatenate([W0[:, 0, C:, :], W0[:, 1, C:, :]],
                            axis=1) / PS                                # [T,128,DOUT]
        wA = wA.transpose(1, 0, 2).reshape(128, T * DOUT).astype(BF)
        wB = np.zeros((BROWS, T, DOUT), np.float32)
        wB[0:64] = W0[:, 2, C:, :].transpose(1, 0, 2) / PS
        for k in range(K):
            for cc in range(C):
                wB[64 + k * C + cc] = W0[:, k, cc, :]
        wB[64 + XR - 1] = bias0
        wB = wB.reshape(BROWS, T * DOUT).astype(BF)

        W1 = mlp(b, inputs["lw1_1"], inputs["lb1_1"], inputs["lw2_1"], inputs["lb2_1"])
        W1 = np.asarray(W1, np.float32).reshape(T, K, DIN1, DOUT) / PS
        w1 = W1.transpose(2, 0, 1, 3).reshape(DIN1, T * K * DOUT).astype(BF)
        bias1 = np.asarray(
            mlp(b, inputs["bw1_1"], inputs["bb1_1"], inputs["bw2_1"], inputs["bb2_1"]),
            np.float32)
        b1 = np.ascontiguousarray(
            np.broadcast_to(bias1.reshape(1, T * DOUT), (128, T * DOUT))).astype(BF)

        c0 = np.ascontiguousarray(
            init_c[0, b][own].reshape(IT, 128, H).transpose(1, 0, 2).reshape(128, IT * H))
        c1 = np.ascontiguousarray(
            init_c[1, b][own].reshape(IT, 128, H).transpose(1, 0, 2).reshape(128, IT * H))

        # fp8 state init: [p, jtp, e, feat]; h1 rows filled (x HS), h0 zero
        s1v0 = np.zeros((JTP, 2, 128, DIN1), np.float32)
        h1o = init_h[1, b][jperm].reshape(JTP, 2, 128, H) * HS
        s1v0[:, :, :, H:DIN1] = h1o
        s1v0 = np.ascontiguousarray(
            s1v0.transpose(2, 0, 1, 3).reshape(128, JTP * 2 * DIN1)).astype(F8)

        in_maps.append({
            "gt8": gt8,
            "s0hA": np.ascontiguousarray(s0hA),
            "s0hB": np.ascontiguousarray(s0hB),
            "wA": np.ascontiguousarray(wA),
            "wB": np.ascontiguousarray(wB),
            "gxb": np.ascontiguousarray(gxb),
            "w1": np.ascontiguousarray(w1),
            "bias1": b1,
            "c0_init": np.ascontiguousarray(c0, np.float32),
            "c1_init": np.ascontiguousarray(c1, np.float32),
            "s1v_init": s1v0,
            "mask64": np.ascontiguousarray(np.broadcast_to(
                np.array([1 - half, half], np.float32).reshape(1, 2),
                (128, 2))),
        })
    return in_maps


def kernel(**inputs) -> np.ndarray:
    global LAST_RESULT
    nc = _build()
    in_maps = _host_prep(inputs)
    res = run_bass_kernel_spmd(nc, in_maps, list(range(NCORES)))
    LAST_RESULT = res

    out = np.zeros((2, L, B, N, H), np.float32)
    for c in range(NCORES):
        b, half = c % 4, c // 4
        o = res.results[c]["out"].reshape(2, L, 128, IT, H)
        out[:, :, b, half * HALF:(half + 1) * HALF, :] = o.transpose(0, 1, 3, 2, 4).reshape(
            2, L, HALF, H)
    return out
